# revision 1
# baseline (speedup 1.0000x reference)
"""Trainium2 Bass kernel for nn_EnhancedGraphConv (gnn_message_passing).

Strategy (8 cores): shard the B*N=1280 graph rows (b,i) as 160 rows/core
(cores 0-3 -> batch 0, 4-7 -> batch 1).  Host converts the dense adjacency
into padded neighbor lists (max degree 51 <= 64 slots/row), and the device
gathers only the ~5% of edge_features / x rows that are actually referenced,
via indirect DMA from HBM.  All per-edge MLPs run on compacted tokens in
feature-major layout; attention softmax runs row-major (rows on partitions,
64 neighbor slots on the free dim); messages are reduced with a PE ones-
broadcast + strided DVE reduction.
"""
import numpy as np
from contextlib import ExitStack

import concourse.bass as bass
import concourse.bacc as bacc
import concourse.tile as tile
from concourse import mybir
from concourse.bass_utils import run_bass_kernel_spmd
from concourse.masks import make_identity

F32 = mybir.dt.float32
I32 = mybir.dt.int32
AF = mybir.ActivationFunctionType
OP = mybir.AluOpType

B, N, C, O, E = 2, 640, 64, 64, 18
D = 64            # neighbor slots per row
RG = 32           # rows per group
NCORES = 8
RPC = (B * N) // NCORES   # 160 rows per core
NG = RPC // RG            # 5 groups
TG = D * RG               # 2048 tokens per group
CHUNK = 512               # matmul moving chunk
NCH = TG // CHUNK         # 4 chunks per group
NCOL = TG // 128          # 16 gather columns per group


def _build_nc(debug=False, stage=6):
    nc = bacc.Bacc("TRN2", target_bir_lowering=False)
    t = {}
    inp = [
        ("efp", [RPC * N, 64]), ("x", [N, C]), ("xrows", [RPC, C]),
        ("am", [RPC, D]),
        ("We1", [E, 64]), ("We2", [64, 64]), ("We3", [64, 32]),
        ("Wpe", [32, 128]), ("Wjj", [64, 128]), ("Wxi", [64, 64]),
        ("Wn", [64, 64]), ("W22", [128, 128]), ("Wa3", [32, 1]),
        ("Ws", [64, 64]), ("Wc1", [128, 64]), ("Wc2", [64, 64]),
        ("be1", [64, 1]), ("be2", [64, 1]), ("be3", [32, 1]),
        ("bhg", [128, 1]), ("bn", [64, 1]), ("ba2", [32, 1]),
        ("bg2", [64, 1]), ("bs", [64, 1]), ("bc1", [64, 1]), ("bc2", [64, 1]),
    ]
    for name, shape in inp:
        t[name] = nc.dram_tensor(name, shape, F32, kind="ExternalInput")
    I16 = mybir.dt.int16
    t["idxj"] = nc.dram_tensor("idxj", [128, NG * (TG // 16)], I16, kind="ExternalInput")
    t["idxe"] = nc.dram_tensor("idxe", [128, NG * (TG // 16)], I16, kind="ExternalInput")
    t["out"] = nc.dram_tensor("out", [RPC, O], F32, kind="ExternalOutput")
    if debug:
        t["dbg_xj"] = nc.dram_tensor("dbg_xj", [128, NCOL, C], F32, kind="ExternalOutput")
        t["dbg_ef"] = nc.dram_tensor("dbg_ef", [128, NCOL, 64], F32, kind="ExternalOutput")
        t["dbg_wflat"] = nc.dram_tensor("dbg_wflat", [1, TG], F32, kind="ExternalOutput")
        t["dbg_mdw"] = nc.dram_tensor("dbg_mdw", [64, TG], F32, kind="ExternalOutput")
        t["dbg_hg"] = nc.dram_tensor("dbg_hg", [128, TG], F32, kind="ExternalOutput")
        t["dbg_wrow"] = nc.dram_tensor("dbg_wrow", [RG, D], F32, kind="ExternalOutput")

    with tile.TileContext(nc) as tc, ExitStack() as ctx:
        w = ctx.enter_context(tc.tile_pool(name="w", bufs=1))
        big = ctx.enter_context(tc.tile_pool(name="big", bufs=1))
        big2 = ctx.enter_context(tc.tile_pool(name="big2", bufs=2))
        sm = ctx.enter_context(tc.tile_pool(name="sm", bufs=2))
        ps = ctx.enter_context(tc.tile_pool(name="ps", bufs=3, space="PSUM"))
        pst = ctx.enter_context(tc.tile_pool(name="pst", bufs=1, space="PSUM"))
        pss = ctx.enter_context(tc.tile_pool(name="pss", bufs=2, space="PSUM"))

        # ---- constants / weights in SBUF
        ident = w.tile([128, 128], F32)
        make_identity(nc, ident[:])
        ones1 = w.tile([1, 64], F32)
        nc.vector.memset(ones1[:], 1.0)
        wt = {}
        for name, shape in inp:
            if name in ("efp", "x", "xrows", "am"):
                continue
            wt[name] = w.tile(shape, F32, name=name)
            nc.sync.dma_start(out=wt[name][:], in_=t[name][:])
        idxj = w.tile([128, NG * (TG // 16)], mybir.dt.int16)
        nc.sync.dma_start(out=idxj[:], in_=t["idxj"][:])
        idxe = w.tile([128, NG * (TG // 16)], mybir.dt.int16)
        nc.sync.dma_start(out=idxe[:], in_=t["idxe"][:])

        # ---- stage 0: per-node precomputes for this core's rows
        xr = w.tile([128, 2, C], F32)   # xrows row-major, chunks of 128/32
        nc.sync.dma_start(out=xr[:, 0, :], in_=t["xrows"][0:128, :])
        nc.sync.dma_start(out=xr[:32, 1, :], in_=t["xrows"][128:160, :])
        xrf = w.tile([C, RPC], F32)     # xrows feature-major
        p0 = pst.tile([C, CHUNK], F32, name="ptr")
        nc.tensor.transpose(p0[:, :128], xr[:, 0, :], ident[:])
        nc.tensor.transpose(p0[:, 128:160], xr[:32, 1, :], ident[:32, :32])
        nc.vector.tensor_copy(out=xrf[:], in_=p0[:, :RPC])
        axi = w.tile([C, RPC], F32)
        pa = pst.tile([C, CHUNK], F32, name="ptr")
        nc.tensor.matmul(pa[:, :RPC], wt["Wxi"][:], xrf[:], start=True, stop=True)
        nc.vector.tensor_copy(out=axi[:], in_=pa[:, :RPC])
        selff = w.tile([C, RPC], F32)
        pb = pst.tile([C, CHUNK], F32, name="ptr")
        nc.tensor.matmul(pb[:, :RPC], wt["Ws"][:], xrf[:], start=True, stop=True)
        nc.scalar.activation(selff[:], pb[:, :RPC], AF.Identity, bias=wt["bs"][:])

        IC = TG // 16   # idx columns per group
        if stage < 6:
            dummy = sm.tile([RG, 64], F32, name="dummy")
            nc.vector.memset(dummy[:], 0.0)
            for g in range(NG):
                nc.sync.dma_start(out=t["out"][g * RG:(g + 1) * RG, :], in_=dummy[:])
        for g in range(NG):
            # ---- gathers (token-major: token t = c*128+p -> [p, c, :])
            xj_tm = big2.tile([128, NCOL, C], F32, name="xj_tm")
            nc.gpsimd.dma_gather(
                out_ap=xj_tm[:], in_ap=t["x"][:],
                idxs_ap=idxj[:, g * IC:(g + 1) * IC],
                num_idxs=TG, num_idxs_reg=TG, elem_size=C,
                single_packet=False)
            ef_tm = big2.tile([128, NCOL, 64], F32, name="ef_tm")
            nc.gpsimd.dma_gather(
                out_ap=ef_tm[:], in_ap=t["efp"][g * RG * N:(g + 1) * RG * N, :],
                idxs_ap=idxe[:, g * IC:(g + 1) * IC],
                num_idxs=TG, num_idxs_reg=TG, elem_size=64,
                single_packet=False)

            if stage < 2:
                continue
            # ---- transpose to feature-major
            xj_fm = big2.tile([C, TG], F32, name="xj_fm")
            ef_fm = big2.tile([E, TG], F32, name="ef_fm")
            for cb in range(NCH):
                ptx = pst.tile([C, CHUNK], F32, name="ptr")
                pte = pst.tile([64, CHUNK], F32, name="pte")
                for k in range(4):
                    c = cb * 4 + k
                    nc.tensor.transpose(ptx[:, k * 128:(k + 1) * 128],
                                        xj_tm[:, c, :], ident[:])
                    nc.tensor.transpose(pte[:, k * 128:(k + 1) * 128],
                                        ef_tm[:, c, :], ident[:])
                cols = slice(cb * CHUNK, (cb + 1) * CHUNK)
                nc.vector.tensor_copy(out=xj_fm[:, cols], in_=ptx[:])
                nc.scalar.activation(ef_fm[:, cols], pte[:E, :], AF.Copy)

            if stage < 3:
                continue
            # ---- per-edge MLPs (feature-major, chunks of 512 tokens)
            pe1 = big.tile([64, TG], F32, name="pe1")
            pe2 = big.tile([64, TG], F32, name="pe2")
            pe3 = big.tile([32, TG], F32, name="pe3")
            hg = big.tile([128, TG], F32, name="hg")
            tn = big.tile([64, TG], F32, name="tn")
            h2 = big.tile([32, TG], F32, name="h2")
            gates = big.tile([64, TG], F32, name="gates")
            # layer-major emission: 4 independent chunks per layer keep PE fed
            for q in range(NCH):
                cols = slice(q * CHUNK, (q + 1) * CHUNK)
                ps1 = ps.tile([128, CHUNK], F32, name="mlp")
                nc.tensor.matmul(ps1[:64, :], wt["We1"][:], ef_fm[:, cols],
                                 start=True, stop=True)
                nc.vector.tensor_scalar(out=pe1[:, cols], in0=ps1[:64, :],
                                        scalar1=wt["be1"][:], scalar2=0.0,
                                        op0=OP.add, op1=OP.max)
            for q in range(NCH):
                cols = slice(q * CHUNK, (q + 1) * CHUNK)
                ps2 = ps.tile([128, CHUNK], F32, name="mlp")
                nc.tensor.matmul(ps2[:64, :], wt["We2"][:], pe1[:, cols],
                                 start=True, stop=True)
                nc.scalar.activation(pe2[:, cols], ps2[:64, :], AF.Relu,
                                     bias=wt["be2"][:])
            for q in range(NCH):
                cols = slice(q * CHUNK, (q + 1) * CHUNK)
                ps3 = ps.tile([128, CHUNK], F32, name="mlp")
                nc.tensor.matmul(ps3[:32, :], wt["We3"][:], pe2[:, cols],
                                 start=True, stop=True)
                nc.vector.tensor_scalar(out=pe3[:, cols], in0=ps3[:32, :],
                                        scalar1=wt["be3"][:], scalar2=0.0,
                                        op0=OP.add, op1=OP.max)
            if stage >= 4:
                axi_b0 = axi[:, g * RG:(g + 1) * RG]
                for q in range(NCH):
                    cols = slice(q * CHUNK, (q + 1) * CHUNK)
                    ps4 = ps.tile([128, CHUNK], F32, name="mlp")
                    nc.tensor.matmul(ps4[:], wt["Wpe"][:], pe3[:, cols],
                                     start=True, stop=False)
                    nc.tensor.matmul(ps4[:], wt["Wjj"][:], xj_fm[:, cols],
                                     start=False, stop=True)
                    axi_b = bass.AP(
                        tensor=axi.tensor,
                        offset=axi_b0.offset,
                        ap=[axi[:].ap[0], [0, CHUNK // RG], [1, RG]])
                    nc.vector.scalar_tensor_tensor(
                        out=ps4[:64, :], in0=ps4[:64, :], scalar=0.0, in1=axi_b,
                        op0=OP.add, op1=OP.add)
                    nc.scalar.activation(hg[:, cols], ps4[:], AF.Relu,
                                         bias=wt["bhg"][:])
                for q in range(NCH):
                    cols = slice(q * CHUNK, (q + 1) * CHUNK)
                    ps5 = ps.tile([128, CHUNK], F32, name="mlp")
                    nc.tensor.matmul(ps5[:64, :], wt["Wn"][:], xj_fm[:, cols],
                                     start=True, stop=True)
                    nc.scalar.activation(tn[:, cols], ps5[:64, :], AF.Identity,
                                         bias=wt["bn"][:])
                for q in range(NCH):
                    cols = slice(q * CHUNK, (q + 1) * CHUNK)
                    ps6 = ps.tile([128, CHUNK], F32, name="mlp")
                    nc.tensor.matmul(ps6[:], wt["W22"][:], hg[:, cols],
                                     start=True, stop=True)
                    nc.vector.tensor_scalar(out=h2[:, cols], in0=ps6[:32, :],
                                            scalar1=wt["ba2"][:], scalar2=0.0,
                                            op0=OP.add, op1=OP.max)
                    nc.scalar.activation(gates[:, cols], ps6[64:128, :],
                                         AF.Sigmoid, bias=wt["bg2"][:])

            if stage < 5:
                continue
            # ---- attention scores: batch 4 d-slots per matmul
            # lhsT = 128 contiguous tokens of h2 -> out[(d%4)*RG + r, d//4]
            psc4 = pss.tile([128, NCOL], F32, name="sp")
            for c in range(NCOL):
                nc.tensor.matmul(psc4[:, c:c + 1],
                                 h2[:, c * 128:(c + 1) * 128],
                                 wt["Wa3"][:], start=True, stop=True)
            sc_rm = sm.tile([RG, D], F32, name="sc_rm")
            for pb in range(4):
                outap = bass.AP(tensor=sc_rm.tensor,
                                offset=sc_rm[:, pb:pb + 1].offset,
                                ap=[sc_rm[:].ap[0], [4, NCOL]])
                nc.vector.tensor_copy(out=outap, in_=psc4[pb * RG:(pb + 1) * RG, :])
            amg = sm.tile([RG, D], F32, name="amg")
            nc.sync.dma_start(out=amg[:], in_=t["am"][g * RG:(g + 1) * RG, :])
            smg = sm.tile([RG, D], F32, name="smg")
            nc.vector.tensor_tensor(out=smg[:], in0=sc_rm[:], in1=amg[:], op=OP.add)
            negmax = sm.tile([RG, 1], F32, name="negmax")
            nc.vector.tensor_reduce(out=negmax[:], in_=smg[:],
                                    axis=mybir.AxisListType.X, op=OP.max,
                                    negate=True)
            pexp = sm.tile([RG, D + 1], F32, name="pexp")
            nc.scalar.activation(pexp[:, :D], smg[:], AF.Exp, bias=negmax[:],
                                 accum_out=pexp[:, D:D + 1])
            invz = sm.tile([RG, 1], F32, name="invz")
            nc.vector.tensor_scalar_add(out=invz[:], in0=pexp[:, D:D + 1],
                                        scalar1=1e-30)
            nc.vector.reciprocal(out=invz[:], in_=invz[:])
            wrow = sm.tile([RG, D], F32, name="wrow")
            nc.vector.tensor_scalar_mul(out=wrow[:], in0=pexp[:, :D],
                                        scalar1=invz[:])
            pwt = pss.tile([D, RG], F32, name="sp")
            nc.tensor.transpose(pwt[:], wrow[:], ident[:RG, :RG])
            wT = sm.tile([D, RG], F32, name="wT")
            nc.vector.tensor_copy(out=wT[:], in_=pwt[:])
            # flatten [D, RG] across partitions into one row [1, TG] (d-major)
            wflat = sm.tile([1, TG], F32, name="wflat")
            nc.sync.dma_start(out=wflat[:], in_=wT[:])

            if stage < 6:
                continue
            # ---- weighted messages
            mdw = big.tile([64, TG], F32, name="mdw")
            for u in range(NCH):
                cols = slice(u * CHUNK, (u + 1) * CHUNK)
                pwb = pst.tile([64, CHUNK], F32, name="pwb")
                nc.tensor.matmul(pwb[:], ones1[:], wflat[:, cols],
                                 start=True, stop=True)
                nc.vector.tensor_tensor(out=mdw[:, cols], in0=gates[:, cols],
                                        in1=tn[:, cols], op=OP.mult)
                nc.vector.tensor_tensor(out=mdw[:, cols], in0=mdw[:, cols],
                                        in1=pwb[:], op=OP.mult)
            if debug and g == 0:
                nc.sync.dma_start(out=t["dbg_xj"][:], in_=xj_tm[:])
                nc.sync.dma_start(out=t["dbg_ef"][:], in_=ef_tm[:])
                nc.sync.dma_start(out=t["dbg_wflat"][:], in_=wflat[:])
                nc.sync.dma_start(out=t["dbg_mdw"][:], in_=mdw[:])
                nc.sync.dma_start(out=t["dbg_hg"][:], in_=hg[:])
                nc.sync.dma_start(out=t["dbg_wrow"][:], in_=wrow[:])
            comb = sm.tile([128, RG], F32, name="comb")
            nc.scalar.activation(comb[:64, :], selff[:, g * RG:(g + 1) * RG],
                                 AF.Copy)
            mdw_v = mdw[:].rearrange("p (d r) -> p r d", d=D)
            nc.vector.tensor_reduce(out=comb[64:128, :], in_=mdw_v,
                                    axis=mybir.AxisListType.X, op=OP.add)

            # ---- output MLP + transpose back to row-major
            pc1 = pss.tile([64, RG], F32, name="sp")
            nc.tensor.matmul(pc1[:], wt["Wc1"][:], comb[:], start=True, stop=True)
            c1 = sm.tile([64, RG], F32, name="c1")
            nc.scalar.activation(c1[:], pc1[:], AF.Relu, bias=wt["bc1"][:])
            pc2 = pss.tile([64, RG], F32, name="sp")
            nc.tensor.matmul(pc2[:], wt["Wc2"][:], c1[:], start=True, stop=True)
            ofm = sm.tile([64, RG], F32, name="ofm")
            nc.scalar.activation(ofm[:], pc2[:], AF.Identity, bias=wt["bc2"][:])
            por = pss.tile([RG, 64], F32, name="sp")
            nc.tensor.transpose(por[:], ofm[:], ident[:64, :64])
            orow = sm.tile([RG, 64], F32, name="orow")
            nc.vector.tensor_copy(out=orow[:], in_=por[:])
            nc.sync.dma_start(out=t["out"][g * RG:(g + 1) * RG, :], in_=orow[:])
    nc.compile()
    return nc


_NC = None


def _host_prep(x, adjacency, edge_features, weights):
    """Build per-core input maps."""
    adj = adjacency > 0
    Bn, Nn = adj.shape[0], adj.shape[1]
    # neighbor lists: stable argsort of ~mask puts nonzero-j first, in order
    order = np.argsort(~adj, axis=-1, kind="stable")   # [B, N, N]
    deg = adj.sum(-1)                                  # [B, N]
    assert deg.max() <= D, f"degree {deg.max()} exceeds {D} slots"
    jidx = order[:, :, :D].astype(np.int32)            # [B, N, D]
    slot = np.arange(D)[None, None, :]
    valid = slot < deg[:, :, None]
    jidx = np.where(valid, jidx, 0)
    am = np.where(valid, 0.0, -1e30).astype(np.float32)  # [B, N, D]

    Wa1, Wg1 = weights["Wa1"], weights["Wg1"]
    W22 = np.zeros((128, 128), np.float32)
    W22[:64, :32] = weights["Wa2"]
    W22[64:, 64:] = weights["Wg2"]
    wts = {
        "We1": weights["We1"], "We2": weights["We2"], "We3": weights["We3"],
        "Wpe": np.concatenate([Wa1[2 * C:], Wg1[C:]], 1),
        "Wjj": np.concatenate([Wa1[C:2 * C], Wg1[:C]], 1),
        "Wxi": Wa1[:C], "Wn": weights["Wn"], "W22": W22,
        "Wa3": weights["Wa3"], "Ws": weights["Ws"],
        "Wc1": weights["Wc1"], "Wc2": weights["Wc2"],
        "be1": weights["be1"][:, None], "be2": weights["be2"][:, None],
        "be3": weights["be3"][:, None],
        "bhg": np.concatenate([weights["ba1"], weights["bg1"]])[:, None],
        "bn": weights["bn"][:, None], "ba2": weights["ba2"][:, None],
        "bg2": weights["bg2"][:, None], "bs": weights["bs"][:, None],
        "bc1": weights["bc1"][:, None], "bc2": weights["bc2"][:, None],
    }
    wts = {k: np.ascontiguousarray(v, np.float32) for k, v in wts.items()}

    in_maps = []
    for core in range(NCORES):
        b = core // 4
        i0 = (core % 4) * RPC
        m = dict(wts)
        m["x"] = np.ascontiguousarray(x[b], np.float32)
        m["xrows"] = np.ascontiguousarray(x[b, i0:i0 + RPC], np.float32)
        efp = np.zeros((RPC * N, 64), np.float32)
        efp[:, :E] = edge_features[b, i0:i0 + RPC].reshape(-1, E)
        m["efp"] = efp
        m["am"] = np.zeros((RPC, D), np.float32)
        IC = TG // 16
        ij = np.zeros((128, NG * IC), np.int16)
        ie = np.zeros((128, NG * IC), np.int16)
        for g in range(NG):
            lr = np.arange(g * RG, (g + 1) * RG)
            m["am"][lr] = am[b, i0 + lr]
            jv = jidx[b, i0 + lr]          # [RG, D]
            # token t = d*RG + rr (gather writes token t to [t%128, t//128])
            jvec = np.zeros(TG, np.int64)
            evec = np.zeros(TG, np.int64)
            for d in range(D):
                tt = d * RG + np.arange(RG)
                jvec[tt] = jv[:, d]
                evec[tt] = (lr - g * RG) * N + jv[:, d]   # group-local row
            # wrapped int16 layout: idx[i%16, i//16], replicated over 8 blocks
            assert evec.max() < 32768
            wj = jvec.reshape(IC, 16).T.astype(np.int16)
            we = evec.reshape(IC, 16).T.astype(np.int16)
            ij[:, g * IC:(g + 1) * IC] = np.tile(wj, (8, 1))
            ie[:, g * IC:(g + 1) * IC] = np.tile(we, (8, 1))
        m["idxj"] = ij
        m["idxe"] = ie
        in_maps.append(m)
    return in_maps


def kernel(**inputs):
    global _NC
    x = np.asarray(inputs["x"], np.float32)
    adjacency = np.asarray(inputs["adjacency"], np.float32)
    edge_features = np.asarray(inputs["edge_features"], np.float32)
    weights = {k: np.asarray(v, np.float32) for k, v in inputs.items()
               if k not in ("x", "adjacency", "edge_features")}
    in_maps = _host_prep(x, adjacency, edge_features, weights)
    if _NC is None:
        _NC = _build_nc()
    res = run_bass_kernel_spmd(_NC, in_maps, list(range(NCORES)))
    out = np.zeros((B, N, O), np.float32)
    for core in range(NCORES):
        b = core // 4
        i0 = (core % 4) * RPC
        out[b, i0:i0 + RPC] = res.results[core]["out"]
    return out



# revision 14
# speedup vs baseline: 3.6284x; 3.6284x over previous
"""Trainium2 Bass kernel for nn_EnhancedGraphConv (gnn_message_passing).

v3: like v2 (host-gathered feature-major bf16 streams, no device gathers)
plus DEGREE-SORTED variable-width groups: each core's 160 rows are sorted
by degree (descending) and grouped in 32s; group g gets only
dg[g] = max-degree-in-group slots (rounded to 4, shared across cores),
cutting padded tokens ~1.67x vs fixed 64 slots.  Gates use the native sigmoid activation table.
"""
import numpy as np
import ml_dtypes
from contextlib import ExitStack

import concourse.bass as bass
import concourse.bacc as bacc
import concourse.tile as tile
from concourse import mybir
from concourse.bass_utils import run_bass_kernel_spmd
from concourse.masks import make_identity

F32 = mybir.dt.float32
BF16 = mybir.dt.bfloat16
NPBF = ml_dtypes.bfloat16
AF = mybir.ActivationFunctionType
OP = mybir.AluOpType

B, N, C, O, E = 2, 640, 64, 64, 18
D = 64            # max neighbor slots per row
RG = 32           # rows per group
NCORES = 8
RPC = (B * N) // NCORES   # 160 rows per core
NG = RPC // RG            # 5 groups
CHUNK = 512               # matmul moving chunk (one PSUM bank)

_WSPEC = [
    ("We1", E, 64), ("We2", 64, 64), ("We3", 64, 32), ("Wpe", 32, 128),
    ("Wjj", 64, 128), ("Wn", 64, 64), ("W22", 128, 128), ("Wa3", 32, 1),
    ("I32r", 32, CHUNK), ("ones32", 32, 64),
]
_FSPEC = [
    ("Ws", 64, 64), ("Wc1", 128, 64), ("Wc2", 64, 64), ("Wxi", 64, 64),
    ("be1", 64, 1), ("be2", 64, 1), ("be3", 32, 1), ("bhg", 128, 1),
    ("bn", 64, 1), ("ba2", 32, 1), ("bg2", 64, 1), ("bs", 64, 1),
    ("bc1", 64, 1), ("bc2", 64, 1),
]


def _layout(spec):
    off, c = {}, 0
    for n, r, w in spec:
        off[n] = (r, c, w)
        c += w
    return off, c


_WOFF, WCOLS = _layout(_WSPEC)
_FOFF, FCOLS = _layout(_FSPEC)


def _build_nc(dgs):
    offs = np.concatenate([[0], np.cumsum([RG * dg for dg in dgs])])
    TOT = int(offs[-1])
    nc = bacc.Bacc("TRN2", target_bir_lowering=False)
    t = {}
    t["wp"] = nc.dram_tensor("wp", [128, WCOLS], BF16, kind="ExternalInput")
    t["fp"] = nc.dram_tensor("fp", [128, FCOLS], F32, kind="ExternalInput")
    t["xj"] = nc.dram_tensor("xj", [C, TOT], BF16, kind="ExternalInput")
    t["ef"] = nc.dram_tensor("ef", [E, TOT], BF16, kind="ExternalInput")
    t["xrf"] = nc.dram_tensor("xrf", [C, RPC], F32, kind="ExternalInput")
    t["am"] = nc.dram_tensor("am", [RPC, D], F32, kind="ExternalInput")
    t["out"] = nc.dram_tensor("out", [RPC, O], F32, kind="ExternalOutput")

    with tile.TileContext(nc) as tc, ExitStack() as ctx:
        w = ctx.enter_context(tc.tile_pool(name="w", bufs=1))
        io = ctx.enter_context(tc.tile_pool(name="io", bufs=3))
        mlp = ctx.enter_context(tc.tile_pool(name="mlp", bufs=3))
        sm = ctx.enter_context(tc.tile_pool(name="sm", bufs=3))
        ps = ctx.enter_context(tc.tile_pool(name="ps", bufs=4, space="PSUM"))
        psc = ctx.enter_context(tc.tile_pool(name="psc", bufs=2, space="PSUM"))
        pss = ctx.enter_context(tc.tile_pool(name="pss", bufs=2, space="PSUM"))

        ident = w.tile([128, 128], F32)
        make_identity(nc, ident[:])
        wp = w.tile([128, WCOLS], BF16, name="wp")
        nc.sync.dma_start(out=wp[:], in_=t["wp"][:])
        fp = w.tile([128, FCOLS], F32, name="fp")
        nc.sync.dma_start(out=fp[:], in_=t["fp"][:])

        def wv(name):
            r, c0, cw = _WOFF[name]
            return wp[:r, c0:c0 + cw]

        def fv(name):
            r, c0, cw = _FOFF[name]
            return fp[:r, c0:c0 + cw]

        # ---- per-node precomputes for this core's rows
        xrf = w.tile([C, RPC], F32, name="xrf")
        nc.sync.dma_start(out=xrf[:], in_=t["xrf"][:])
        pa = pss.tile([C, RPC], F32, name="sp")
        nc.tensor.matmul(pa[:], fv("Wxi"), xrf[:], start=True, stop=True)
        axi = w.tile([C, RPC], F32, name="axi")
        nc.vector.tensor_copy(out=axi[:], in_=pa[:])
        pb = pss.tile([C, RPC], F32, name="sp")
        nc.tensor.matmul(pb[:], fv("Ws"), xrf[:], start=True, stop=True)
        selff = w.tile([C, RPC], F32, name="selff")
        nc.scalar.activation(selff[:], pb[:], AF.Identity, bias=fv("bs"))
        # transposed per-group axi: axiT[:, g*C:(g+1)*C] = axi[:, gRG:(g+1)RG].T
        axiT = w.tile([RG, NG * C], BF16, name="axiT")
        for g in range(NG):
            pt = pss.tile([RG, C], F32, name="sp")
            nc.tensor.transpose(pt[:], axi[:, g * RG:(g + 1) * RG],
                                ident[:C, :C])
            nc.vector.tensor_copy(out=axiT[:, g * C:(g + 1) * C], in_=pt[:])

        def emit_tail(st):
            g, dg, TGg, chunks = st["g"], st["dg"], st["TGg"], st["chunks"]
            tn, eg, sc_rm = st["tn"], st["eg"], st["sc_rm"]
            amg = sm.tile([RG, dg], F32, name="amg")
            nc.sync.dma_start(out=amg[:],
                              in_=t["am"][g * RG:(g + 1) * RG, :dg])
            smg = sm.tile([RG, dg], F32, name="smg")
            nc.vector.tensor_tensor(out=smg[:], in0=sc_rm[:], in1=amg[:],
                                    op=OP.add)
            negmax = sm.tile([RG, 1], F32, name="negmax")
            nc.vector.tensor_reduce(out=negmax[:], in_=smg[:],
                                    axis=mybir.AxisListType.X, op=OP.max,
                                    negate=True)
            pexp = sm.tile([RG, dg + 1], F32, name="pexp")
            nc.scalar.activation(pexp[:, :dg], smg[:], AF.Exp, bias=negmax[:],
                                 accum_out=pexp[:, dg:dg + 1])
            invz = sm.tile([RG, 1], F32, name="invz")
            nc.vector.tensor_scalar_add(out=invz[:], in0=pexp[:, dg:dg + 1],
                                        scalar1=1e-30)
            nc.vector.reciprocal(out=invz[:], in_=invz[:])
            wrow = sm.tile([RG, dg], BF16, name="wrow")
            nc.vector.tensor_scalar_mul(out=wrow[:], in0=pexp[:, :dg],
                                        scalar1=invz[:])
            # rhsq[r, t] = wrow[r, t//32 + off] * delta(t%32 == r); then
            # pwb = ones32.T @ rhsq broadcasts w(t) over feature partitions.
            mdw = mlp.tile([64, TGg], F32, name="mdw")
            rhsq = sm.tile([RG, TGg], BF16, name="rhsq")
            for s, e in chunks:
                cols = slice(s, e)
                nd = (e - s) // RG
                wb_ap = bass.AP(
                    tensor=wrow.tensor,
                    offset=wrow[:, s // RG:s // RG + nd].offset,
                    ap=[wrow[:].ap[0], [1, nd], [0, RG]])
                nc.vector.tensor_tensor(out=rhsq[:, cols], in0=wb_ap,
                                        in1=wv("I32r")[:, :e - s], op=OP.mult)
                pwb = ps.tile([128, CHUNK], F32, name="mlp")
                nc.tensor.matmul(pwb[:64, :e - s], wv("ones32"),
                                 rhsq[:, cols], start=True, stop=True)
                nc.vector.tensor_tensor(out=mdw[:, cols], in0=tn[:, cols],
                                        in1=pwb[:64, :e - s], op=OP.mult)
                nc.vector.tensor_tensor(out=mdw[:, cols], in0=mdw[:, cols],
                                        in1=eg[:, cols], op=OP.mult)
            comb = sm.tile([128, RG], F32, name="comb")
            nc.vector.tensor_copy(out=comb[:64, :],
                                  in_=selff[:, g * RG:(g + 1) * RG])
            mdw_v = mdw[:].rearrange("p (d r) -> p r d", d=dg)
            nc.vector.tensor_reduce(out=comb[64:128, :], in_=mdw_v,
                                    axis=mybir.AxisListType.X, op=OP.add)
            # output MLP + transpose back to row-major
            pc1 = pss.tile([64, RG], F32, name="sp")
            nc.tensor.matmul(pc1[:], fv("Wc1"), comb[:], start=True, stop=True)
            c1 = sm.tile([64, RG], F32, name="c1")
            nc.vector.tensor_scalar(out=c1[:], in0=pc1[:],
                                    scalar1=fv("bc1"), scalar2=0.0,
                                    op0=OP.add, op1=OP.max)
            pc2 = pss.tile([64, RG], F32, name="sp")
            nc.tensor.matmul(pc2[:], fv("Wc2"), c1[:], start=True, stop=True)
            ofm = sm.tile([64, RG], F32, name="ofm")
            nc.vector.tensor_scalar_add(out=ofm[:], in0=pc2[:],
                                        scalar1=fv("bc2"))
            por = pss.tile([RG, 64], F32, name="sp")
            nc.tensor.transpose(por[:], ofm[:], ident[:64, :64])
            orow = sm.tile([RG, 64], F32, name="orow")
            nc.vector.tensor_copy(out=orow[:], in_=por[:])
            nc.gpsimd.dma_start(out=t["out"][g * RG:(g + 1) * RG, :],
                                in_=orow[:])

        carry = None
        for g in range(NG):
            dg = dgs[g]
            TGg = RG * dg
            chunks = [(s, min(s + CHUNK, TGg)) for s in range(0, TGg, CHUNK)]
            gc = slice(int(offs[g]), int(offs[g + 1]))
            xjg = io.tile([C, TGg], BF16, name="xjg")
            nc.gpsimd.dma_start(out=xjg[:], in_=t["xj"][:, gc])
            efg = io.tile([E, TGg], BF16, name="efg")
            nc.sync.dma_start(out=efg[:], in_=t["ef"][:, gc])

            pe1 = mlp.tile([64, TGg], BF16, name="pe1")
            pe2 = mlp.tile([64, TGg], BF16, name="pe2")
            pe3 = mlp.tile([32, TGg], BF16, name="pe3")
            hg = mlp.tile([128, TGg], BF16, name="hg")
            tn = mlp.tile([64, TGg], F32, name="tn")
            eg = mlp.tile([64, TGg], F32, name="eg")
            h2 = mlp.tile([32, TGg], BF16, name="h2")

            for s, e in chunks:
                cols = slice(s, e)
                ps1 = ps.tile([128, CHUNK], F32, name="mlp")
                nc.tensor.matmul(ps1[:64, :e - s], wv("We1"), efg[:, cols],
                                 start=True, stop=True)
                nc.vector.tensor_scalar(out=pe1[:, cols], in0=ps1[:64, :e - s],
                                        scalar1=fv("be1"), scalar2=0.0,
                                        op0=OP.add, op1=OP.max)
            for s, e in chunks:
                cols = slice(s, e)
                ps2 = ps.tile([128, CHUNK], F32, name="mlp")
                nc.tensor.matmul(ps2[:64, :e - s], wv("We2"), pe1[:, cols],
                                 start=True, stop=True)
                nc.scalar.activation(pe2[:, cols], ps2[:64, :e - s], AF.Relu,
                                     bias=fv("be2"))
            for s, e in chunks:
                cols = slice(s, e)
                ps3 = ps.tile([128, CHUNK], F32, name="mlp")
                nc.tensor.matmul(ps3[:32, :e - s], wv("We3"), pe2[:, cols],
                                 start=True, stop=True)
                nc.vector.tensor_scalar(out=pe3[:, cols], in0=ps3[:32, :e - s],
                                        scalar1=fv("be3"), scalar2=0.0,
                                        op0=OP.add, op1=OP.max)
            # hg = relu(Wpe@pe3 + Wjj@xj + axi(row) + bhg) ; axi added via
            # matmul with replicated 32-identity rhs selecting the row block.
            axiTg = axiT[:, g * C:(g + 1) * C]
            for s, e in chunks:
                cols = slice(s, e)
                ps4 = ps.tile([128, CHUNK], F32, name="mlp")
                nc.tensor.matmul(ps4[:, :e - s], wv("Wpe"), pe3[:, cols],
                                 start=True, stop=False)
                nc.tensor.matmul(ps4[:64, :e - s], axiTg,
                                 wv("I32r")[:, :e - s],
                                 start=False, stop=False)
                nc.tensor.matmul(ps4[:, :e - s], wv("Wjj"), xjg[:, cols],
                                 start=False, stop=True)
                nc.scalar.activation(hg[:, cols], ps4[:, :e - s], AF.Relu,
                                     bias=fv("bhg"))
            for s, e in chunks:
                cols = slice(s, e)
                ps5 = ps.tile([128, CHUNK], F32, name="mlp")
                nc.tensor.matmul(ps5[:64, :e - s], wv("Wn"), xjg[:, cols],
                                 start=True, stop=True)
                nc.scalar.activation(tn[:, cols], ps5[:64, :e - s],
                                     AF.Identity, bias=fv("bn"))
            # h2 = relu(Wa2.T@h1 + ba2); gates = sigmoid(Wg2.T@g1 + bg2)
            for s, e in chunks:
                cols = slice(s, e)
                ps6 = ps.tile([128, CHUNK], F32, name="mlp")
                nc.tensor.matmul(ps6[:, :e - s], wv("W22"), hg[:, cols],
                                 start=True, stop=True)
                nc.scalar.activation(h2[:, cols], ps6[:32, :e - s], AF.Relu,
                                     bias=fv("ba2"))
                nc.scalar.activation(eg[:, cols], ps6[64:128, :e - s],
                                     AF.Sigmoid, bias=fv("bg2"))

            # ---- attention scores: lhsT = 128 contiguous tokens of h2
            nsc = TGg // 128
            psc4 = psc.tile([128, nsc], F32, name="sc")
            for c in range(nsc):
                nc.tensor.matmul(psc4[:, c:c + 1],
                                 h2[:, c * 128:(c + 1) * 128],
                                 wv("Wa3"), start=True, stop=True)
            sc_rm = sm.tile([RG, dg], F32, name="sc_rm")
            for pb4 in range(4):
                outap = bass.AP(tensor=sc_rm.tensor,
                                offset=sc_rm[:, pb4:pb4 + 1].offset,
                                ap=[sc_rm[:].ap[0], [4, nsc]])
                nc.vector.tensor_copy(out=outap,
                                      in_=psc4[pb4 * RG:(pb4 + 1) * RG, :])

            st = dict(g=g, dg=dg, TGg=TGg, chunks=chunks,
                      tn=tn, eg=eg, sc_rm=sc_rm)
            if carry is not None:
                emit_tail(carry)
            carry = st
        emit_tail(carry)
    nc.compile()
    return nc


_NC = None
_NC_KEY = None


def _host_prep(x, adjacency, edge_features, weights):
    """Build per-core input maps (sort by degree, gather + pack on host).

    Returns (in_maps, perms, dgs): perms[core] maps sorted position ->
    local row index within the core's 160 rows.
    """
    adj = adjacency > 0
    order = np.argsort(~adj, axis=-1, kind="stable")   # [B, N, N]
    deg = adj.sum(-1)                                  # [B, N]
    assert deg.max() <= D, f"degree {deg.max()} exceeds {D} slots"
    jidx = order[:, :, :D].astype(np.int64)            # [B, N, D]
    slot = np.arange(D)[None, None, :]
    valid = slot < deg[:, :, None]
    jidx = np.where(valid, jidx, 0)
    am = np.where(valid, 0.0, -1e30).astype(np.float32)  # [B, N, D]

    # per-core degree-descending row order; shared per-group slot widths
    perms = []
    dgs = np.zeros(NG, np.int64)
    for core in range(NCORES):
        b = core // 4
        i0 = (core % 4) * RPC
        p = np.argsort(-deg[b, i0:i0 + RPC], kind="stable")
        perms.append(p)
        sd = deg[b, i0:i0 + RPC][p]
        for g in range(NG):
            mx = int(sd[g * RG:(g + 1) * RG].max())
            dgs[g] = max(dgs[g], -(-mx // 4) * 4, 4)
    dgs = [int(v) for v in dgs]
    offs = np.concatenate([[0], np.cumsum([RG * dg for dg in dgs])])

    Wa1, Wg1 = weights["Wa1"], weights["Wg1"]
    W22 = np.zeros((128, 128), np.float32)
    W22[:64, :32] = weights["Wa2"]
    W22[64:, 64:] = weights["Wg2"]
    wvals = {
        "We1": weights["We1"], "We2": weights["We2"], "We3": weights["We3"],
        "Wpe": np.concatenate([Wa1[2 * C:], Wg1[C:]], 1),
        "Wjj": np.concatenate([Wa1[C:2 * C], Wg1[:C]], 1),
        "Wn": weights["Wn"], "W22": W22, "Wa3": weights["Wa3"],
        "I32r": np.tile(np.eye(RG, dtype=np.float32), (1, CHUNK // RG)),
        "ones32": np.ones((RG, 64), np.float32),
    }
    fvals = {
        "Wxi": Wa1[:C], "Ws": weights["Ws"],
        "Wc1": weights["Wc1"], "Wc2": weights["Wc2"],
        "be1": weights["be1"][:, None], "be2": weights["be2"][:, None],
        "be3": weights["be3"][:, None],
        "bhg": np.concatenate([weights["ba1"], weights["bg1"]])[:, None],
        "bn": weights["bn"][:, None], "ba2": weights["ba2"][:, None],
        "bg2": weights["bg2"][:, None], "bs": weights["bs"][:, None],
        "bc1": weights["bc1"][:, None], "bc2": weights["bc2"][:, None],
    }
    wpk = np.zeros((128, WCOLS), NPBF)
    for name, (r, c0, cw) in _WOFF.items():
        v = np.asarray(wvals[name], np.float32)
        assert v.shape == (r, cw), (name, v.shape, (r, cw))
        wpk[:r, c0:c0 + cw] = v.astype(NPBF)
    fpk = np.zeros((128, FCOLS), np.float32)
    for name, (r, c0, cw) in _FOFF.items():
        v = np.asarray(fvals[name], np.float32)
        assert v.shape == (r, cw), (name, v.shape, (r, cw))
        fpk[:r, c0:c0 + cw] = v

    TOT = int(offs[-1])
    in_maps = []
    for core in range(NCORES):
        b = core // 4
        i0 = (core % 4) * RPC
        p = perms[core]
        jv = jidx[b, i0:i0 + RPC][p]                   # [RPC, D] sorted rows
        # token col = offs[g] + d*RG + r  (d-major per group, d < dgs[g])
        jcol = np.zeros(TOT, np.int64)
        lrow = np.zeros(TOT, np.int64)                 # sorted-local row
        for g in range(NG):
            dg = dgs[g]
            blk = jv[g * RG:(g + 1) * RG, :dg]         # [RG, dg]
            jcol[offs[g]:offs[g + 1]] = blk.T.reshape(-1)
            lr = np.broadcast_to(np.arange(g * RG, (g + 1) * RG)[None, :],
                                 (dg, RG)).reshape(-1)
            lrow[offs[g]:offs[g + 1]] = lr
        grow = i0 + p[lrow]                            # global row in batch b
        m = {
            "wp": wpk, "fp": fpk,
            "xj": np.ascontiguousarray(x[b].T[:, jcol].astype(NPBF)),
            "ef": np.ascontiguousarray(
                edge_features[b, grow, jcol, :].T.astype(NPBF)),
            "xrf": np.ascontiguousarray(x[b, i0:i0 + RPC][p].T, np.float32),
            "am": np.ascontiguousarray(am[b, i0:i0 + RPC][p], np.float32),
        }
        in_maps.append(m)
    return in_maps, perms, dgs


def kernel(**inputs):
    global _NC, _NC_KEY
    x = np.asarray(inputs["x"], np.float32)
    adjacency = np.asarray(inputs["adjacency"], np.float32)
    edge_features = np.asarray(inputs["edge_features"], np.float32)
    weights = {k: np.asarray(v, np.float32) for k, v in inputs.items()
               if k not in ("x", "adjacency", "edge_features")}
    in_maps, perms, dgs = _host_prep(x, adjacency, edge_features, weights)
    key = tuple(dgs)
    if _NC is None or _NC_KEY != key:
        _NC = _build_nc(dgs)
        _NC_KEY = key
    res = run_bass_kernel_spmd(_NC, in_maps, list(range(NCORES)))
    out = np.zeros((B, N, O), np.float32)
    for core in range(NCORES):
        b = core // 4
        i0 = (core % 4) * RPC
        out[b, i0 + perms[core]] = res.results[core]["out"]
    return out


# revision 15
# speedup vs baseline: 12.0663x; 3.3255x over previous
"""Trainium2 Bass kernel for nn_EnhancedGraphConv (gnn_message_passing).

v3: like v2 (host-gathered feature-major bf16 streams, no device gathers)
plus DEGREE-SORTED variable-width groups: each core's 160 rows are sorted
by degree (descending) and grouped in 32s; group g gets only
dg[g] = max-degree-in-group slots (rounded to 4, shared across cores),
cutting padded tokens ~1.67x vs fixed 64 slots.  Gates use the native sigmoid activation table.
"""
import numpy as np
import ml_dtypes
from contextlib import ExitStack

import concourse.bass as bass
import concourse.bacc as bacc
import concourse.tile as tile
from concourse import mybir
from concourse.bass_utils import run_bass_kernel_spmd
from concourse.masks import make_identity

F32 = mybir.dt.float32
BF16 = mybir.dt.bfloat16
NPBF = ml_dtypes.bfloat16
AF = mybir.ActivationFunctionType
OP = mybir.AluOpType

B, N, C, O, E = 2, 640, 64, 64, 18
D = 64            # max neighbor slots per row
RG = 32           # rows per group
NCORES = 8
RPC = (B * N) // NCORES   # 160 rows per core
NG = RPC // RG            # 5 groups
CHUNK = 512               # matmul moving chunk (one PSUM bank)

_WSPEC = [
    ("We1", E, 64), ("We2", 64, 64), ("We3", 64, 32), ("Wpe", 32, 128),
    ("Wjj", 64, 128), ("Wn", 64, 64), ("W22", 128, 128), ("Wa3", 32, 1),
    ("I32r", 32, CHUNK), ("ones32", 32, 64),
]
_FSPEC = [
    ("Ws", 64, 64), ("Wc1", 128, 64), ("Wc2", 64, 64), ("Wxi", 64, 64),
    ("be1", 64, 1), ("be2", 64, 1), ("be3", 32, 1), ("bhg", 128, 1),
    ("bn", 64, 1), ("ba2", 32, 1), ("bg2", 64, 1), ("bs", 64, 1),
    ("bc1", 64, 1), ("bc2", 64, 1),
]


def _layout(spec):
    off, c = {}, 0
    for n, r, w in spec:
        off[n] = (r, c, w)
        c += w
    return off, c


_WOFF, WCOLS = _layout(_WSPEC)
_FOFF, FCOLS = _layout(_FSPEC)


def _build_nc(dgs):
    offs = np.concatenate([[0], np.cumsum([RG * dg for dg in dgs])])
    TOT = int(offs[-1])
    nc = bacc.Bacc("TRN2", target_bir_lowering=False)
    t = {}
    t["wp"] = nc.dram_tensor("wp", [128, WCOLS], BF16, kind="ExternalInput")
    t["fp"] = nc.dram_tensor("fp", [128, FCOLS], F32, kind="ExternalInput")
    t["xj"] = nc.dram_tensor("xj", [C, TOT], BF16, kind="ExternalInput")
    t["ef"] = nc.dram_tensor("ef", [E, TOT], BF16, kind="ExternalInput")
    t["xrf"] = nc.dram_tensor("xrf", [C, RPC], F32, kind="ExternalInput")
    t["am"] = nc.dram_tensor("am", [RPC, D], F32, kind="ExternalInput")
    t["out"] = nc.dram_tensor("out", [RPC, O], F32, kind="ExternalOutput")

    with tile.TileContext(nc) as tc, ExitStack() as ctx:
        w = ctx.enter_context(tc.tile_pool(name="w", bufs=1))
        io = ctx.enter_context(tc.tile_pool(name="io", bufs=3))
        mlp = ctx.enter_context(tc.tile_pool(name="mlp", bufs=3))
        sm = ctx.enter_context(tc.tile_pool(name="sm", bufs=3))
        ps = ctx.enter_context(tc.tile_pool(name="ps", bufs=4, space="PSUM"))
        psc = ctx.enter_context(tc.tile_pool(name="psc", bufs=2, space="PSUM"))
        pss = ctx.enter_context(tc.tile_pool(name="pss", bufs=2, space="PSUM"))

        ident = w.tile([128, 128], F32)
        make_identity(nc, ident[:])
        wp = w.tile([128, WCOLS], BF16, name="wp")
        nc.sync.dma_start(out=wp[:], in_=t["wp"][:])
        fp = w.tile([128, FCOLS], F32, name="fp")
        nc.sync.dma_start(out=fp[:], in_=t["fp"][:])

        def wv(name):
            r, c0, cw = _WOFF[name]
            return wp[:r, c0:c0 + cw]

        def fv(name):
            r, c0, cw = _FOFF[name]
            return fp[:r, c0:c0 + cw]

        # ---- per-node precomputes for this core's rows
        xrf = w.tile([C, RPC], F32, name="xrf")
        nc.sync.dma_start(out=xrf[:], in_=t["xrf"][:])
        pa = pss.tile([C, RPC], F32, name="sp")
        nc.tensor.matmul(pa[:], fv("Wxi"), xrf[:], start=True, stop=True)
        axi = w.tile([C, RPC], F32, name="axi")
        nc.vector.tensor_copy(out=axi[:], in_=pa[:])
        pb = pss.tile([C, RPC], F32, name="sp")
        nc.tensor.matmul(pb[:], fv("Ws"), xrf[:], start=True, stop=True)
        selff = w.tile([C, RPC], F32, name="selff")
        nc.scalar.activation(selff[:], pb[:], AF.Identity, bias=fv("bs"))
        # transposed per-group axi: axiT[:, g*C:(g+1)*C] = axi[:, gRG:(g+1)RG].T
        axiT = w.tile([RG, NG * C], BF16, name="axiT")
        for g in range(NG):
            pt = pss.tile([RG, C], F32, name="sp")
            nc.tensor.transpose(pt[:], axi[:, g * RG:(g + 1) * RG],
                                ident[:C, :C])
            nc.vector.tensor_copy(out=axiT[:, g * C:(g + 1) * C], in_=pt[:])

        def emit_tail(st):
            g, dg, TGg, chunks = st["g"], st["dg"], st["TGg"], st["chunks"]
            tn, eg, sc_rm = st["tn"], st["eg"], st["sc_rm"]
            amg = sm.tile([RG, dg], F32, name="amg")
            nc.sync.dma_start(out=amg[:],
                              in_=t["am"][g * RG:(g + 1) * RG, :dg])
            smg = sm.tile([RG, dg], F32, name="smg")
            nc.vector.tensor_tensor(out=smg[:], in0=sc_rm[:], in1=amg[:],
                                    op=OP.add)
            negmax = sm.tile([RG, 1], F32, name="negmax")
            nc.vector.tensor_reduce(out=negmax[:], in_=smg[:],
                                    axis=mybir.AxisListType.X, op=OP.max,
                                    negate=True)
            pexp = sm.tile([RG, dg + 1], F32, name="pexp")
            nc.scalar.activation(pexp[:, :dg], smg[:], AF.Exp, bias=negmax[:],
                                 accum_out=pexp[:, dg:dg + 1])
            invz = sm.tile([RG, 1], F32, name="invz")
            nc.vector.tensor_scalar_add(out=invz[:], in0=pexp[:, dg:dg + 1],
                                        scalar1=1e-30)
            nc.vector.reciprocal(out=invz[:], in_=invz[:])
            wrow = sm.tile([RG, dg], BF16, name="wrow")
            nc.vector.tensor_scalar_mul(out=wrow[:], in0=pexp[:, :dg],
                                        scalar1=invz[:])
            # rhsq[r, t] = wrow[r, t//32 + off] * delta(t%32 == r); then
            # pwb = ones32.T @ rhsq broadcasts w(t) over feature partitions.
            mdw = mlp.tile([64, TGg], F32, name="mdw")
            rhsq = sm.tile([RG, TGg], BF16, name="rhsq")
            for s, e in chunks:
                cols = slice(s, e)
                nd = (e - s) // RG
                wb_ap = bass.AP(
                    tensor=wrow.tensor,
                    offset=wrow[:, s // RG:s // RG + nd].offset,
                    ap=[wrow[:].ap[0], [1, nd], [0, RG]])
                nc.vector.tensor_tensor(out=rhsq[:, cols], in0=wb_ap,
                                        in1=wv("I32r")[:, :e - s], op=OP.mult)
                pwb = ps.tile([128, CHUNK], F32, name="mlp")
                nc.tensor.matmul(pwb[:64, :e - s], wv("ones32"),
                                 rhsq[:, cols], start=True, stop=True)
                nc.vector.tensor_tensor(out=mdw[:, cols], in0=tn[:, cols],
                                        in1=pwb[:64, :e - s], op=OP.mult)
                nc.vector.tensor_tensor(out=mdw[:, cols], in0=mdw[:, cols],
                                        in1=eg[:, cols], op=OP.mult)
            comb = sm.tile([128, RG], F32, name="comb")
            nc.vector.tensor_copy(out=comb[:64, :],
                                  in_=selff[:, g * RG:(g + 1) * RG])
            mdw_v = mdw[:].rearrange("p (d r) -> p r d", d=dg)
            nc.vector.tensor_reduce(out=comb[64:128, :], in_=mdw_v,
                                    axis=mybir.AxisListType.X, op=OP.add)
            # output MLP + transpose back to row-major
            pc1 = pss.tile([64, RG], F32, name="sp")
            nc.tensor.matmul(pc1[:], fv("Wc1"), comb[:], start=True, stop=True)
            c1 = sm.tile([64, RG], F32, name="c1")
            nc.vector.tensor_scalar(out=c1[:], in0=pc1[:],
                                    scalar1=fv("bc1"), scalar2=0.0,
                                    op0=OP.add, op1=OP.max)
            pc2 = pss.tile([64, RG], F32, name="sp")
            nc.tensor.matmul(pc2[:], fv("Wc2"), c1[:], start=True, stop=True)
            ofm = sm.tile([64, RG], F32, name="ofm")
            nc.vector.tensor_scalar_add(out=ofm[:], in0=pc2[:],
                                        scalar1=fv("bc2"))
            por = pss.tile([RG, 64], F32, name="sp")
            nc.tensor.transpose(por[:], ofm[:], ident[:64, :64])
            orow = sm.tile([RG, 64], F32, name="orow")
            nc.scalar.activation(orow[:], por[:], AF.Copy)
            nc.gpsimd.dma_start(out=t["out"][g * RG:(g + 1) * RG, :],
                                in_=orow[:])

        carry = None
        for g in range(NG):
            dg = dgs[g]
            TGg = RG * dg
            chunks = [(s, min(s + CHUNK, TGg)) for s in range(0, TGg, CHUNK)]
            gc = slice(int(offs[g]), int(offs[g + 1]))
            xjg = io.tile([C, TGg], BF16, name="xjg")
            nc.gpsimd.dma_start(out=xjg[:], in_=t["xj"][:, gc])
            efg = io.tile([E, TGg], BF16, name="efg")
            nc.sync.dma_start(out=efg[:], in_=t["ef"][:, gc])

            pe1 = mlp.tile([64, TGg], BF16, name="pe1")
            pe2 = mlp.tile([64, TGg], BF16, name="pe2")
            pe3 = mlp.tile([32, TGg], BF16, name="pe3")
            hg = mlp.tile([128, TGg], BF16, name="hg")
            tn = mlp.tile([64, TGg], F32, name="tn")
            eg = mlp.tile([64, TGg], F32, name="eg")
            h2 = mlp.tile([32, TGg], BF16, name="h2")

            for s, e in chunks:
                cols = slice(s, e)
                ps1 = ps.tile([128, CHUNK], F32, name="mlp")
                nc.tensor.matmul(ps1[:64, :e - s], wv("We1"), efg[:, cols],
                                 start=True, stop=True)
                nc.vector.tensor_scalar(out=pe1[:, cols], in0=ps1[:64, :e - s],
                                        scalar1=fv("be1"), scalar2=0.0,
                                        op0=OP.add, op1=OP.max)
            for s, e in chunks:
                cols = slice(s, e)
                ps2 = ps.tile([128, CHUNK], F32, name="mlp")
                nc.tensor.matmul(ps2[:64, :e - s], wv("We2"), pe1[:, cols],
                                 start=True, stop=True)
                nc.scalar.activation(pe2[:, cols], ps2[:64, :e - s], AF.Relu,
                                     bias=fv("be2"))
            for s, e in chunks:
                cols = slice(s, e)
                ps3 = ps.tile([128, CHUNK], F32, name="mlp")
                nc.tensor.matmul(ps3[:32, :e - s], wv("We3"), pe2[:, cols],
                                 start=True, stop=True)
                nc.vector.tensor_scalar(out=pe3[:, cols], in0=ps3[:32, :e - s],
                                        scalar1=fv("be3"), scalar2=0.0,
                                        op0=OP.add, op1=OP.max)
            # hg = relu(Wpe@pe3 + Wjj@xj + axi(row) + bhg) ; axi added via
            # matmul with replicated 32-identity rhs selecting the row block.
            axiTg = axiT[:, g * C:(g + 1) * C]
            for s, e in chunks:
                cols = slice(s, e)
                ps4 = ps.tile([128, CHUNK], F32, name="mlp")
                nc.tensor.matmul(ps4[:, :e - s], wv("Wpe"), pe3[:, cols],
                                 start=True, stop=False)
                nc.tensor.matmul(ps4[:64, :e - s], axiTg,
                                 wv("I32r")[:, :e - s],
                                 start=False, stop=False)
                nc.tensor.matmul(ps4[:, :e - s], wv("Wjj"), xjg[:, cols],
                                 start=False, stop=True)
                nc.scalar.activation(hg[:, cols], ps4[:, :e - s], AF.Relu,
                                     bias=fv("bhg"))
            for s, e in chunks:
                cols = slice(s, e)
                ps5 = ps.tile([128, CHUNK], F32, name="mlp")
                nc.tensor.matmul(ps5[:64, :e - s], wv("Wn"), xjg[:, cols],
                                 start=True, stop=True)
                nc.scalar.activation(tn[:, cols], ps5[:64, :e - s],
                                     AF.Identity, bias=fv("bn"))
            # h2 = relu(Wa2.T@h1 + ba2); gates = sigmoid(Wg2.T@g1 + bg2)
            for s, e in chunks:
                cols = slice(s, e)
                ps6 = ps.tile([128, CHUNK], F32, name="mlp")
                nc.tensor.matmul(ps6[:, :e - s], wv("W22"), hg[:, cols],
                                 start=True, stop=True)
                nc.scalar.activation(h2[:, cols], ps6[:32, :e - s], AF.Relu,
                                     bias=fv("ba2"))
                nc.scalar.activation(eg[:, cols], ps6[64:128, :e - s],
                                     AF.Sigmoid, bias=fv("bg2"))

            # ---- attention scores: lhsT = 128 contiguous tokens of h2
            nsc = TGg // 128
            psc4 = psc.tile([128, nsc], F32, name="sc")
            for c in range(nsc):
                nc.tensor.matmul(psc4[:, c:c + 1],
                                 h2[:, c * 128:(c + 1) * 128],
                                 wv("Wa3"), start=True, stop=True)
            sc_rm = sm.tile([RG, dg], F32, name="sc_rm")
            for pb4 in range(4):
                outap = bass.AP(tensor=sc_rm.tensor,
                                offset=sc_rm[:, pb4:pb4 + 1].offset,
                                ap=[sc_rm[:].ap[0], [4, nsc]])
                nc.scalar.activation(outap,
                                     psc4[pb4 * RG:(pb4 + 1) * RG, :],
                                     AF.Copy)

            st = dict(g=g, dg=dg, TGg=TGg, chunks=chunks,
                      tn=tn, eg=eg, sc_rm=sc_rm)
            if carry is not None:
                emit_tail(carry)
            carry = st
        emit_tail(carry)
    nc.compile()
    return nc


_NC = None
_NC_KEY = None


def _host_prep(x, adjacency, edge_features, weights):
    """Build per-core input maps (sort by degree, gather + pack on host).

    Returns (in_maps, perms, dgs): perms[core] maps sorted position ->
    local row index within the core's 160 rows.
    """
    adj = adjacency > 0
    order = np.argsort(~adj, axis=-1, kind="stable")   # [B, N, N]
    deg = adj.sum(-1)                                  # [B, N]
    assert deg.max() <= D, f"degree {deg.max()} exceeds {D} slots"
    jidx = order[:, :, :D].astype(np.int64)            # [B, N, D]
    slot = np.arange(D)[None, None, :]
    valid = slot < deg[:, :, None]
    jidx = np.where(valid, jidx, 0)
    am = np.where(valid, 0.0, -1e30).astype(np.float32)  # [B, N, D]

    # per-core degree-descending row order; shared per-group slot widths
    perms = []
    dgs = np.zeros(NG, np.int64)
    for core in range(NCORES):
        b = core // 4
        i0 = (core % 4) * RPC
        p = np.argsort(-deg[b, i0:i0 + RPC], kind="stable")
        perms.append(p)
        sd = deg[b, i0:i0 + RPC][p]
        for g in range(NG):
            mx = int(sd[g * RG:(g + 1) * RG].max())
            dgs[g] = max(dgs[g], -(-mx // 4) * 4, 4)
    dgs = [int(v) for v in dgs]
    offs = np.concatenate([[0], np.cumsum([RG * dg for dg in dgs])])

    Wa1, Wg1 = weights["Wa1"], weights["Wg1"]
    W22 = np.zeros((128, 128), np.float32)
    W22[:64, :32] = weights["Wa2"]
    W22[64:, 64:] = weights["Wg2"]
    wvals = {
        "We1": weights["We1"], "We2": weights["We2"], "We3": weights["We3"],
        "Wpe": np.concatenate([Wa1[2 * C:], Wg1[C:]], 1),
        "Wjj": np.concatenate([Wa1[C:2 * C], Wg1[:C]], 1),
        "Wn": weights["Wn"], "W22": W22, "Wa3": weights["Wa3"],
        "I32r": np.tile(np.eye(RG, dtype=np.float32), (1, CHUNK // RG)),
        "ones32": np.ones((RG, 64), np.float32),
    }
    fvals = {
        "Wxi": Wa1[:C], "Ws": weights["Ws"],
        "Wc1": weights["Wc1"], "Wc2": weights["Wc2"],
        "be1": weights["be1"][:, None], "be2": weights["be2"][:, None],
        "be3": weights["be3"][:, None],
        "bhg": np.concatenate([weights["ba1"], weights["bg1"]])[:, None],
        "bn": weights["bn"][:, None], "ba2": weights["ba2"][:, None],
        "bg2": weights["bg2"][:, None], "bs": weights["bs"][:, None],
        "bc1": weights["bc1"][:, None], "bc2": weights["bc2"][:, None],
    }
    wpk = np.zeros((128, WCOLS), NPBF)
    for name, (r, c0, cw) in _WOFF.items():
        v = np.asarray(wvals[name], np.float32)
        assert v.shape == (r, cw), (name, v.shape, (r, cw))
        wpk[:r, c0:c0 + cw] = v.astype(NPBF)
    fpk = np.zeros((128, FCOLS), np.float32)
    for name, (r, c0, cw) in _FOFF.items():
        v = np.asarray(fvals[name], np.float32)
        assert v.shape == (r, cw), (name, v.shape, (r, cw))
        fpk[:r, c0:c0 + cw] = v

    TOT = int(offs[-1])
    in_maps = []
    for core in range(NCORES):
        b = core // 4
        i0 = (core % 4) * RPC
        p = perms[core]
        jv = jidx[b, i0:i0 + RPC][p]                   # [RPC, D] sorted rows
        # token col = offs[g] + d*RG + r  (d-major per group, d < dgs[g])
        jcol = np.zeros(TOT, np.int64)
        lrow = np.zeros(TOT, np.int64)                 # sorted-local row
        for g in range(NG):
            dg = dgs[g]
            blk = jv[g * RG:(g + 1) * RG, :dg]         # [RG, dg]
            jcol[offs[g]:offs[g + 1]] = blk.T.reshape(-1)
            lr = np.broadcast_to(np.arange(g * RG, (g + 1) * RG)[None, :],
                                 (dg, RG)).reshape(-1)
            lrow[offs[g]:offs[g + 1]] = lr
        grow = i0 + p[lrow]                            # global row in batch b
        m = {
            "wp": wpk, "fp": fpk,
            "xj": np.ascontiguousarray(x[b].T[:, jcol].astype(NPBF)),
            "ef": np.ascontiguousarray(
                edge_features[b, grow, jcol, :].T.astype(NPBF)),
            "xrf": np.ascontiguousarray(x[b, i0:i0 + RPC][p].T, np.float32),
            "am": np.ascontiguousarray(am[b, i0:i0 + RPC][p], np.float32),
        }
        in_maps.append(m)
    return in_maps, perms, dgs


def kernel(**inputs):
    global _NC, _NC_KEY
    x = np.asarray(inputs["x"], np.float32)
    adjacency = np.asarray(inputs["adjacency"], np.float32)
    edge_features = np.asarray(inputs["edge_features"], np.float32)
    weights = {k: np.asarray(v, np.float32) for k, v in inputs.items()
               if k not in ("x", "adjacency", "edge_features")}
    in_maps, perms, dgs = _host_prep(x, adjacency, edge_features, weights)
    key = tuple(dgs)
    if _NC is None or _NC_KEY != key:
        _NC = _build_nc(dgs)
        _NC_KEY = key
    res = run_bass_kernel_spmd(_NC, in_maps, list(range(NCORES)))
    out = np.zeros((B, N, O), np.float32)
    for core in range(NCORES):
        b = core // 4
        i0 = (core % 4) * RPC
        out[b, i0 + perms[core]] = res.results[core]["out"]
    return out


# revision 16
# speedup vs baseline: 13.8349x; 1.1466x over previous
"""Trainium2 Bass kernel for nn_EnhancedGraphConv (gnn_message_passing).

v3: like v2 (host-gathered feature-major bf16 streams, no device gathers)
plus DEGREE-SORTED variable-width groups: each core's 160 rows are sorted
by degree (descending) and grouped in 32s; group g gets only
dg[g] = max-degree-in-group slots (rounded to 4, shared across cores),
cutting padded tokens ~1.67x vs fixed 64 slots.  Gates use the native sigmoid activation table.
"""
import numpy as np
import ml_dtypes
from contextlib import ExitStack

import concourse.bass as bass
import concourse.bacc as bacc
import concourse.tile as tile
from concourse import mybir
from concourse.bass_utils import run_bass_kernel_spmd
from concourse.masks import make_identity

F32 = mybir.dt.float32
BF16 = mybir.dt.bfloat16
NPBF = ml_dtypes.bfloat16
AF = mybir.ActivationFunctionType
OP = mybir.AluOpType

B, N, C, O, E = 2, 640, 64, 64, 18
D = 64            # max neighbor slots per row
RG = 32           # rows per group
NCORES = 8
RPC = (B * N) // NCORES   # 160 rows per core
NG = RPC // RG            # 5 groups
CHUNK = 512               # matmul moving chunk (one PSUM bank)

_WSPEC = [
    ("We1", E, 64), ("We2", 64, 64), ("We3", 64, 32), ("Wpe", 32, 128),
    ("Wjj", 64, 128), ("Wn", 64, 64), ("W22", 128, 128), ("Wa3", 32, 1),
    ("I32r", 32, CHUNK), ("ones32", 32, 64),
]
_FSPEC = [
    ("Ws", 64, 64), ("Wc1", 128, 64), ("Wc2", 64, 64), ("Wxi", 64, 64),
    ("be1", 64, 1), ("be2", 64, 1), ("be3", 32, 1), ("bhg", 128, 1),
    ("bn", 64, 1), ("ba2", 32, 1), ("bg2", 64, 1), ("bs", 64, 1),
    ("bc1", 64, 1), ("bc2", 64, 1),
]


def _layout(spec):
    off, c = {}, 0
    for n, r, w in spec:
        off[n] = (r, c, w)
        c += w
    return off, c


_WOFF, WCOLS = _layout(_WSPEC)
_FOFF, FCOLS = _layout(_FSPEC)


def _build_nc(dgs):
    offs = np.concatenate([[0], np.cumsum([RG * dg for dg in dgs])])
    TOT = int(offs[-1])
    nc = bacc.Bacc("TRN2", target_bir_lowering=False)
    t = {}
    t["wp"] = nc.dram_tensor("wp", [128, WCOLS], BF16, kind="ExternalInput")
    t["fp"] = nc.dram_tensor("fp", [128, FCOLS], F32, kind="ExternalInput")
    t["xj"] = nc.dram_tensor("xj", [C, TOT], BF16, kind="ExternalInput")
    t["ef"] = nc.dram_tensor("ef", [E, TOT], BF16, kind="ExternalInput")
    t["xrf"] = nc.dram_tensor("xrf", [C, RPC], F32, kind="ExternalInput")
    t["am"] = nc.dram_tensor("am", [RPC, D], F32, kind="ExternalInput")
    t["out"] = nc.dram_tensor("out", [RPC, O], F32, kind="ExternalOutput")

    with tile.TileContext(nc) as tc, ExitStack() as ctx:
        w = ctx.enter_context(tc.tile_pool(name="w", bufs=1))
        io = ctx.enter_context(tc.tile_pool(name="io", bufs=3))
        mlp = ctx.enter_context(tc.tile_pool(name="mlp", bufs=3))
        sm = ctx.enter_context(tc.tile_pool(name="sm", bufs=3))
        ps = ctx.enter_context(tc.tile_pool(name="ps", bufs=4, space="PSUM"))
        psc = ctx.enter_context(tc.tile_pool(name="psc", bufs=2, space="PSUM"))
        pss = ctx.enter_context(tc.tile_pool(name="pss", bufs=2, space="PSUM"))

        ident = w.tile([128, 128], F32)
        make_identity(nc, ident[:])
        wp = w.tile([128, WCOLS], BF16, name="wp")
        nc.sync.dma_start(out=wp[:], in_=t["wp"][:])
        fp = w.tile([128, FCOLS], F32, name="fp")
        nc.sync.dma_start(out=fp[:], in_=t["fp"][:])

        def wv(name):
            r, c0, cw = _WOFF[name]
            return wp[:r, c0:c0 + cw]

        def fv(name):
            r, c0, cw = _FOFF[name]
            return fp[:r, c0:c0 + cw]

        xrf = w.tile([C, RPC], F32, name="xrf")
        nc.sync.dma_start(out=xrf[:], in_=t["xrf"][:])
        axi = w.tile([C, RPC], F32, name="axi")
        selff = w.tile([C, RPC], F32, name="selff")
        axiT = w.tile([RG, NG * C], BF16, name="axiT")

        def emit_precompute():
            # per-node precomputes; emitted after group 0's first layers so
            # the engines start on bulk work immediately
            pa = pss.tile([C, RPC], F32, name="sp")
            nc.tensor.matmul(pa[:], fv("Wxi"), xrf[:], start=True, stop=True)
            nc.vector.tensor_copy(out=axi[:], in_=pa[:])
            pb = pss.tile([C, RPC], F32, name="sp")
            nc.tensor.matmul(pb[:], fv("Ws"), xrf[:], start=True, stop=True)
            nc.scalar.activation(selff[:], pb[:], AF.Identity, bias=fv("bs"))
            for gg in range(NG):
                pt = pss.tile([RG, C], F32, name="sp")
                nc.tensor.transpose(pt[:], axi[:, gg * RG:(gg + 1) * RG],
                                    ident[:C, :C])
                nc.vector.tensor_copy(out=axiT[:, gg * C:(gg + 1) * C],
                                      in_=pt[:])

        def emit_tail(st):
            g, dg, TGg, chunks = st["g"], st["dg"], st["TGg"], st["chunks"]
            tn, eg, sc_rm = st["tn"], st["eg"], st["sc_rm"]
            amg = sm.tile([RG, dg], F32, name="amg")
            nc.sync.dma_start(out=amg[:],
                              in_=t["am"][g * RG:(g + 1) * RG, :dg])
            smg = sm.tile([RG, dg], F32, name="smg")
            nc.vector.tensor_tensor(out=smg[:], in0=sc_rm[:], in1=amg[:],
                                    op=OP.add)
            negmax = sm.tile([RG, 1], F32, name="negmax")
            nc.vector.tensor_reduce(out=negmax[:], in_=smg[:],
                                    axis=mybir.AxisListType.X, op=OP.max,
                                    negate=True)
            pexp = sm.tile([RG, dg + 1], F32, name="pexp")
            nc.scalar.activation(pexp[:, :dg], smg[:], AF.Exp, bias=negmax[:],
                                 accum_out=pexp[:, dg:dg + 1])
            invz = sm.tile([RG, 1], F32, name="invz")
            nc.vector.tensor_scalar_add(out=invz[:], in0=pexp[:, dg:dg + 1],
                                        scalar1=1e-30)
            nc.vector.reciprocal(out=invz[:], in_=invz[:])
            wrow = sm.tile([RG, dg], BF16, name="wrow")
            nc.vector.tensor_scalar_mul(out=wrow[:], in0=pexp[:, :dg],
                                        scalar1=invz[:])
            # rhsq[r, t] = wrow[r, t//32 + off] * delta(t%32 == r); then
            # pwb = ones32.T @ rhsq broadcasts w(t) over feature partitions.
            mdw = mlp.tile([64, TGg], F32, name="mdw")
            rhsq = sm.tile([RG, TGg], BF16, name="rhsq")
            for s, e in chunks:
                cols = slice(s, e)
                nd = (e - s) // RG
                wb_ap = bass.AP(
                    tensor=wrow.tensor,
                    offset=wrow[:, s // RG:s // RG + nd].offset,
                    ap=[wrow[:].ap[0], [1, nd], [0, RG]])
                nc.vector.tensor_tensor(out=rhsq[:, cols], in0=wb_ap,
                                        in1=wv("I32r")[:, :e - s], op=OP.mult)
                pwb = ps.tile([128, CHUNK], F32, name="mlp")
                nc.tensor.matmul(pwb[:64, :e - s], wv("ones32"),
                                 rhsq[:, cols], start=True, stop=True)
                nc.vector.tensor_tensor(out=mdw[:, cols], in0=tn[:, cols],
                                        in1=pwb[:64, :e - s], op=OP.mult)
                nc.vector.tensor_tensor(out=mdw[:, cols], in0=mdw[:, cols],
                                        in1=eg[:, cols], op=OP.mult)
            comb = sm.tile([128, RG], F32, name="comb")
            nc.vector.tensor_copy(out=comb[:64, :],
                                  in_=selff[:, g * RG:(g + 1) * RG])
            mdw_v = mdw[:].rearrange("p (d r) -> p r d", d=dg)
            nc.vector.tensor_reduce(out=comb[64:128, :], in_=mdw_v,
                                    axis=mybir.AxisListType.X, op=OP.add)
            # output MLP + transpose back to row-major
            pc1 = pss.tile([64, RG], F32, name="sp")
            nc.tensor.matmul(pc1[:], fv("Wc1"), comb[:], start=True, stop=True)
            c1 = sm.tile([64, RG], F32, name="c1")
            nc.vector.tensor_scalar(out=c1[:], in0=pc1[:],
                                    scalar1=fv("bc1"), scalar2=0.0,
                                    op0=OP.add, op1=OP.max)
            pc2 = pss.tile([64, RG], F32, name="sp")
            nc.tensor.matmul(pc2[:], fv("Wc2"), c1[:], start=True, stop=True)
            ofm = sm.tile([64, RG], F32, name="ofm")
            nc.vector.tensor_scalar_add(out=ofm[:], in0=pc2[:],
                                        scalar1=fv("bc2"))
            por = pss.tile([RG, 64], F32, name="sp")
            nc.tensor.transpose(por[:], ofm[:], ident[:64, :64])
            orow = sm.tile([RG, 64], F32, name="orow")
            nc.scalar.activation(orow[:], por[:], AF.Copy)
            nc.gpsimd.dma_start(out=t["out"][g * RG:(g + 1) * RG, :],
                                in_=orow[:])

        carry = None
        for g in range(NG):
            dg = dgs[g]
            TGg = RG * dg
            chunks = [(s, min(s + CHUNK, TGg)) for s in range(0, TGg, CHUNK)]
            gc = slice(int(offs[g]), int(offs[g + 1]))
            xjg = io.tile([C, TGg], BF16, name="xjg")
            nc.gpsimd.dma_start(out=xjg[:], in_=t["xj"][:, gc])
            efg = io.tile([E, TGg], BF16, name="efg")
            nc.sync.dma_start(out=efg[:], in_=t["ef"][:, gc])

            pe1 = mlp.tile([64, TGg], BF16, name="pe1")
            pe2 = mlp.tile([64, TGg], BF16, name="pe2")
            pe3 = mlp.tile([32, TGg], BF16, name="pe3")
            hg = mlp.tile([128, TGg], BF16, name="hg")
            tn = mlp.tile([64, TGg], F32, name="tn")
            eg = mlp.tile([64, TGg], F32, name="eg")
            h2 = mlp.tile([32, TGg], BF16, name="h2")

            for s, e in chunks:
                cols = slice(s, e)
                ps1 = ps.tile([128, CHUNK], F32, name="mlp")
                nc.tensor.matmul(ps1[:64, :e - s], wv("We1"), efg[:, cols],
                                 start=True, stop=True)
                nc.vector.tensor_scalar(out=pe1[:, cols], in0=ps1[:64, :e - s],
                                        scalar1=fv("be1"), scalar2=0.0,
                                        op0=OP.add, op1=OP.max)
            if g == 0:
                emit_precompute()
            for s, e in chunks:
                cols = slice(s, e)
                ps2 = ps.tile([128, CHUNK], F32, name="mlp")
                nc.tensor.matmul(ps2[:64, :e - s], wv("We2"), pe1[:, cols],
                                 start=True, stop=True)
                nc.scalar.activation(pe2[:, cols], ps2[:64, :e - s], AF.Relu,
                                     bias=fv("be2"))
            for s, e in chunks:
                cols = slice(s, e)
                ps3 = ps.tile([128, CHUNK], F32, name="mlp")
                nc.tensor.matmul(ps3[:32, :e - s], wv("We3"), pe2[:, cols],
                                 start=True, stop=True)
                nc.vector.tensor_scalar(out=pe3[:, cols], in0=ps3[:32, :e - s],
                                        scalar1=fv("be3"), scalar2=0.0,
                                        op0=OP.add, op1=OP.max)
            # hg = relu(Wpe@pe3 + Wjj@xj + axi(row) + bhg) ; axi added via
            # matmul with replicated 32-identity rhs selecting the row block.
            axiTg = axiT[:, g * C:(g + 1) * C]
            for s, e in chunks:
                cols = slice(s, e)
                ps4 = ps.tile([128, CHUNK], F32, name="mlp")
                nc.tensor.matmul(ps4[:, :e - s], wv("Wpe"), pe3[:, cols],
                                 start=True, stop=False)
                nc.tensor.matmul(ps4[:64, :e - s], axiTg,
                                 wv("I32r")[:, :e - s],
                                 start=False, stop=False)
                nc.tensor.matmul(ps4[:, :e - s], wv("Wjj"), xjg[:, cols],
                                 start=False, stop=True)
                nc.scalar.activation(hg[:, cols], ps4[:, :e - s], AF.Relu,
                                     bias=fv("bhg"))
            for s, e in chunks:
                cols = slice(s, e)
                ps5 = ps.tile([128, CHUNK], F32, name="mlp")
                nc.tensor.matmul(ps5[:64, :e - s], wv("Wn"), xjg[:, cols],
                                 start=True, stop=True)
                nc.scalar.activation(tn[:, cols], ps5[:64, :e - s],
                                     AF.Identity, bias=fv("bn"))
            # h2 = relu(Wa2.T@h1 + ba2); gates = sigmoid(Wg2.T@g1 + bg2)
            for s, e in chunks:
                cols = slice(s, e)
                ps6 = ps.tile([128, CHUNK], F32, name="mlp")
                nc.tensor.matmul(ps6[:, :e - s], wv("W22"), hg[:, cols],
                                 start=True, stop=True)
                nc.scalar.activation(h2[:, cols], ps6[:32, :e - s], AF.Relu,
                                     bias=fv("ba2"))
                nc.scalar.activation(eg[:, cols], ps6[64:128, :e - s],
                                     AF.Sigmoid, bias=fv("bg2"))

            # ---- attention scores: lhsT = 128 contiguous tokens of h2
            nsc = TGg // 128
            psc4 = psc.tile([128, nsc], F32, name="sc")
            for c in range(nsc):
                nc.tensor.matmul(psc4[:, c:c + 1],
                                 h2[:, c * 128:(c + 1) * 128],
                                 wv("Wa3"), start=True, stop=True)
            sc_rm = sm.tile([RG, dg], F32, name="sc_rm")
            for pb4 in range(4):
                outap = bass.AP(tensor=sc_rm.tensor,
                                offset=sc_rm[:, pb4:pb4 + 1].offset,
                                ap=[sc_rm[:].ap[0], [4, nsc]])
                nc.scalar.activation(outap,
                                     psc4[pb4 * RG:(pb4 + 1) * RG, :],
                                     AF.Copy)

            st = dict(g=g, dg=dg, TGg=TGg, chunks=chunks,
                      tn=tn, eg=eg, sc_rm=sc_rm)
            if carry is not None:
                emit_tail(carry)
            carry = st
        emit_tail(carry)
    nc.compile()
    return nc


_NC = None
_NC_KEY = None


def _host_prep(x, adjacency, edge_features, weights):
    """Build per-core input maps (sort by degree, gather + pack on host).

    Returns (in_maps, perms, dgs): perms[core] maps sorted position ->
    local row index within the core's 160 rows.
    """
    adj = adjacency > 0
    order = np.argsort(~adj, axis=-1, kind="stable")   # [B, N, N]
    deg = adj.sum(-1)                                  # [B, N]
    assert deg.max() <= D, f"degree {deg.max()} exceeds {D} slots"
    jidx = order[:, :, :D].astype(np.int64)            # [B, N, D]
    slot = np.arange(D)[None, None, :]
    valid = slot < deg[:, :, None]
    jidx = np.where(valid, jidx, 0)
    am = np.where(valid, 0.0, -1e30).astype(np.float32)  # [B, N, D]

    # per-core degree-descending row order; shared per-group slot widths
    perms = []
    dgs = np.zeros(NG, np.int64)
    for core in range(NCORES):
        b = core // 4
        i0 = (core % 4) * RPC
        p = np.argsort(-deg[b, i0:i0 + RPC], kind="stable")
        perms.append(p)
        sd = deg[b, i0:i0 + RPC][p]
        for g in range(NG):
            mx = int(sd[g * RG:(g + 1) * RG].max())
            dgs[g] = max(dgs[g], -(-mx // 4) * 4, 4)
    dgs = [int(v) for v in dgs]
    offs = np.concatenate([[0], np.cumsum([RG * dg for dg in dgs])])

    Wa1, Wg1 = weights["Wa1"], weights["Wg1"]
    W22 = np.zeros((128, 128), np.float32)
    W22[:64, :32] = weights["Wa2"]
    W22[64:, 64:] = weights["Wg2"]
    wvals = {
        "We1": weights["We1"], "We2": weights["We2"], "We3": weights["We3"],
        "Wpe": np.concatenate([Wa1[2 * C:], Wg1[C:]], 1),
        "Wjj": np.concatenate([Wa1[C:2 * C], Wg1[:C]], 1),
        "Wn": weights["Wn"], "W22": W22, "Wa3": weights["Wa3"],
        "I32r": np.tile(np.eye(RG, dtype=np.float32), (1, CHUNK // RG)),
        "ones32": np.ones((RG, 64), np.float32),
    }
    fvals = {
        "Wxi": Wa1[:C], "Ws": weights["Ws"],
        "Wc1": weights["Wc1"], "Wc2": weights["Wc2"],
        "be1": weights["be1"][:, None], "be2": weights["be2"][:, None],
        "be3": weights["be3"][:, None],
        "bhg": np.concatenate([weights["ba1"], weights["bg1"]])[:, None],
        "bn": weights["bn"][:, None], "ba2": weights["ba2"][:, None],
        "bg2": weights["bg2"][:, None], "bs": weights["bs"][:, None],
        "bc1": weights["bc1"][:, None], "bc2": weights["bc2"][:, None],
    }
    wpk = np.zeros((128, WCOLS), NPBF)
    for name, (r, c0, cw) in _WOFF.items():
        v = np.asarray(wvals[name], np.float32)
        assert v.shape == (r, cw), (name, v.shape, (r, cw))
        wpk[:r, c0:c0 + cw] = v.astype(NPBF)
    fpk = np.zeros((128, FCOLS), np.float32)
    for name, (r, c0, cw) in _FOFF.items():
        v = np.asarray(fvals[name], np.float32)
        assert v.shape == (r, cw), (name, v.shape, (r, cw))
        fpk[:r, c0:c0 + cw] = v

    TOT = int(offs[-1])
    in_maps = []
    for core in range(NCORES):
        b = core // 4
        i0 = (core % 4) * RPC
        p = perms[core]
        jv = jidx[b, i0:i0 + RPC][p]                   # [RPC, D] sorted rows
        # token col = offs[g] + d*RG + r  (d-major per group, d < dgs[g])
        jcol = np.zeros(TOT, np.int64)
        lrow = np.zeros(TOT, np.int64)                 # sorted-local row
        for g in range(NG):
            dg = dgs[g]
            blk = jv[g * RG:(g + 1) * RG, :dg]         # [RG, dg]
            jcol[offs[g]:offs[g + 1]] = blk.T.reshape(-1)
            lr = np.broadcast_to(np.arange(g * RG, (g + 1) * RG)[None, :],
                                 (dg, RG)).reshape(-1)
            lrow[offs[g]:offs[g + 1]] = lr
        grow = i0 + p[lrow]                            # global row in batch b
        m = {
            "wp": wpk, "fp": fpk,
            "xj": np.ascontiguousarray(x[b].T[:, jcol].astype(NPBF)),
            "ef": np.ascontiguousarray(
                edge_features[b, grow, jcol, :].T.astype(NPBF)),
            "xrf": np.ascontiguousarray(x[b, i0:i0 + RPC][p].T, np.float32),
            "am": np.ascontiguousarray(am[b, i0:i0 + RPC][p], np.float32),
        }
        in_maps.append(m)
    return in_maps, perms, dgs


def kernel(**inputs):
    global _NC, _NC_KEY
    x = np.asarray(inputs["x"], np.float32)
    adjacency = np.asarray(inputs["adjacency"], np.float32)
    edge_features = np.asarray(inputs["edge_features"], np.float32)
    weights = {k: np.asarray(v, np.float32) for k, v in inputs.items()
               if k not in ("x", "adjacency", "edge_features")}
    in_maps, perms, dgs = _host_prep(x, adjacency, edge_features, weights)
    key = tuple(dgs)
    if _NC is None or _NC_KEY != key:
        _NC = _build_nc(dgs)
        _NC_KEY = key
    res = run_bass_kernel_spmd(_NC, in_maps, list(range(NCORES)))
    out = np.zeros((B, N, O), np.float32)
    for core in range(NCORES):
        b = core // 4
        i0 = (core % 4) * RPC
        out[b, i0 + perms[core]] = res.results[core]["out"]
    return out
